# revision 16
# baseline (speedup 1.0000x reference)
"""Channel self-attention (inverted-energy softmax) Trainium2 Bass kernel.

Computes, for x: [B, C, P] (B=32, C=256, P=8192), alpha: [1]:
    energy    = x @ x.T                     (per sample, [C, C])
    inv       = rowmax(energy) - energy
    attention = softmax(inv, axis=-1)
    out       = alpha * (attention @ x) + x

Sharding: pure data-parallel over B across 8 NeuronCores (4 samples/core).

Math notes:
  softmax(rowmax(E) - E) row i == exp(m_i - E[i,j]) / Z_i with
  m_i = rowmin_j E[i,j]  (shift-invariance; matches jax's exponent),
  Z_i = sum_j exp(m_i - E[i,j]).
  The whole epilogue folds into one matmul:
    out = (W + I) @ x,  W[i,j] = (alpha / Z_i) * exp(m_i - E[i,j])
  so the residual add and normalization cost nothing on the vector
  engines — the PSUM result only needs a plain copy to SBUF staging.
  At alpha=0 (the shipped fill) W == 0 and out == bf16(x) exactly.

Perf design (v3 — the problem sits at the DMA/PE ridge):
  HBM traffic is the binding roofline. Loads read fp32 x (mandatory
  32 MiB/core) and cast to bf16 *during* the DMA (SWDGE gpsimd path);
  x lives in SBUF only as bf16 (xn, triple-buffered so loads never
  stall the energy phase of the next-next sample). The output is
  written as bf16 (16 MiB/core) and widened to fp32 on the host: bf16
  keeps the fp32 exponent range, so rounding is uniformly <= 2^-9
  relative — far inside the 2e-2 gate even for denormal-range elements
  (fp16 fails there). Per-core DMA floor ~141 us @ 358 GB/s.

  PE work/sample (target ~125 us/core, just under the DMA floor):
    x-chunk transposes      16384 cyc
    energy matmuls          24576 cyc  (symmetric: E00/E01/E11 only,
                             E10 = E01^T via one fp32 PE transpose 512)
    W^T transposes            512 cyc
    out = (W+I) @ x         32768 cyc
  Emission is a cross-sample software pipeline: sample b's output
  phase interleaves with sample b+1's energy phase, keeping PE, the
  copy engines (DVE/ACT alternate) and both DMA rings (SWDGE loads,
  HWDGE stores) concurrently fed.
"""

from contextlib import ExitStack

import numpy as np

import concourse.bass as bass
import concourse.tile as tile
from concourse import bacc, mybir
from concourse.bass_utils import run_bass_kernel_spmd
from concourse.masks import make_identity

F32 = mybir.dt.float32
BF16 = mybir.dt.bfloat16

N_CORES = 8
FULL_B, C, P = 32, 256, 8192


def build(nsamp, c, p):
    """Build + compile the per-core Bass program: x [nsamp, c, p] -> out."""
    assert c == 256, "kernel hardcodes C=256 (two 128-partition halves)"
    assert p % 4096 == 0
    kc = p // 128          # contraction chunks for the energy matmul
    nunits = kc // 4       # transpose/matmul units (512 cols each)
    nout = p // 1024       # 1024-wide output column chunks
    stg_w = 4096           # output staging width (bf16 -> 1 MiB DMAs)
    nst = stg_w // 1024
    ldw = 2048             # input DMA chunk width (fp32 -> 1 MiB reads)

    nc = bacc.Bacc("TRN2", target_bir_lowering=False, debug=False)
    x_d = nc.dram_tensor("x", [nsamp, c, p], F32, kind="ExternalInput").ap()
    a_d = nc.dram_tensor("alpha", [1], F32, kind="ExternalInput").ap()
    o_d = nc.dram_tensor("out", [nsamp, c, p], BF16, kind="ExternalOutput").ap()

    with tile.TileContext(nc) as tc, ExitStack() as ctx:
        consts = ctx.enter_context(tc.tile_pool(name="consts", bufs=1))
        xnpool = ctx.enter_context(tc.tile_pool(name="xn", bufs=3))
        xtpool = ctx.enter_context(tc.tile_pool(name="xt", bufs=4))
        upool = ctx.enter_context(tc.tile_pool(name="u", bufs=2))
        wpool = ctx.enter_context(tc.tile_pool(name="w", bufs=2))
        utpool = ctx.enter_context(tc.tile_pool(name="ut", bufs=2))
        vpool = ctx.enter_context(tc.tile_pool(name="vec", bufs=4))
        opool = ctx.enter_context(tc.tile_pool(name="ostg", bufs=2))
        tp_psum = ctx.enter_context(tc.tile_pool(name="tp", bufs=2, space="PSUM"))
        e_psum = ctx.enter_context(tc.tile_pool(name="e", bufs=1, space="PSUM"))
        o_psum = ctx.enter_context(tc.tile_pool(name="o", bufs=2, space="PSUM"))

        def emit_load(b):
            # fp32 HBM -> bf16 SBUF, cast inside the SWDGE DMA engines.
            # Sample 0 uses finer chunks so the energy phase starts on
            # the first arrival instead of waiting a full 1 MiB read.
            st = {"b": b, "xn": []}
            for h in range(2):
                t = xnpool.tile([128, p], BF16, tag=f"xn{h}", name=f"xn{h}")
                st["xn"].append(t)
            w = ldw // 2 if b == 0 else ldw
            for ch in range(p // w):
                for h in range(2):
                    nc.gpsimd.dma_start(
                        out=st["xn"][h][:, ch * w:(ch + 1) * w],
                        in_=x_d[b, h * 128:(h + 1) * 128,
                                ch * w:(ch + 1) * w],
                    )
            return st

        # identity first (0.3us of gpsimd), then sample 0's loads so the
        # SWDGE queue starts the pipeline immediately; alpha follows.
        ident = consts.tile([128, 128], F32)
        make_identity(nc, ident)
        identb = consts.tile([128, 128], BF16)
        nc.gpsimd.tensor_copy(out=identb[:], in_=ident[:])

        st_cur = emit_load(0)

        alpha_b = consts.tile([128, 1], F32)
        nc.gpsimd.dma_start(out=alpha_b, in_=a_d.to_broadcast([128, 1]))

        # ~3.5us of throwaway PE transposes during the first-load DMA
        # latency: keeps the PE_HAM activity window busy so the clock
        # gate is already released (2.4 GHz) when real work arrives.
        warm_ps = tp_psum.tile([128, 4 * c], BF16, tag="tp", name="warm")
        for _ in range(30):
            nc.tensor.transpose(warm_ps[:, 0:128], identb[:], identb[:])

        def energy_gen(st):
            """Yields after each 4-chunk unit (transposes one unit ahead).

            Symmetric-energy: per chunk only E00, E01, E11 accumulate
            (3 matmuls, 128-wide each); E10 is recovered after the loop
            as E01^T (emit_softmax_ut).
            """
            xn = st["xn"]
            st["e_ps"] = [
                e_psum.tile([128, c], F32, tag=f"e{h}", name=f"e{h}")
                for h in range(2)
            ]

            def emit_trans(kp2):
                # one unit = 4 contraction chunks (512 cols): 8 PE
                # transposes into a single one-bank PSUM tile
                # ([128,1024] bf16 = 2KB/partition), one wide copy out.
                tp = tp_psum.tile([128, 4 * c], BF16, tag="tp", name="tp")
                for u4 in range(4):
                    k = kp2 * 4 + u4
                    for h in range(2):
                        nc.tensor.transpose(
                            tp[:, u4 * c + h * 128:u4 * c + (h + 1) * 128],
                            xn[h][:, k * 128:(k + 1) * 128],
                            identb[:],
                        )
                xt = xtpool.tile([128, 4 * c], BF16, tag="xt", name="xt")
                # all on DVE: its 0.69us/copy beats the 1.07us PE unit
                # cadence, while ACT's 1.11us would gate it.
                nc.vector.tensor_copy(out=xt[:], in_=tp[:])
                return xt

            def emit_emm(kp2, xt):
                # one accumulation group per PSUM bank: full rows for the
                # top half (E00|E01), E11 only for the bottom (E10 is
                # recovered as E01^T afterwards).
                e0, e1 = st["e_ps"]
                for u4 in range(4):
                    k = 4 * kp2 + u4
                    x0 = xt[:, u4 * c:u4 * c + 128]
                    x1 = xt[:, u4 * c + 128:u4 * c + 256]
                    kw = dict(start=(k == 0), stop=(k == kc - 1))
                    nc.tensor.matmul(
                        e0[:], lhsT=x0, rhs=xt[:, u4 * c:(u4 + 1) * c], **kw
                    )
                    nc.tensor.matmul(e1[:, 128:256], lhsT=x1, rhs=x1, **kw)

            xt_prev = emit_trans(0)
            yield
            for kp2 in range(1, nunits):
                xt_cur = emit_trans(kp2)
                emit_emm(kp2 - 1, xt_prev)
                xt_prev = xt_cur
                yield
            emit_emm(nunits - 1, xt_prev)

        def emit_softmax_ut(st):
            e0, e1 = st["e_ps"]

            # E10 = E01^T: one DVE copy out of PSUM + one fp32 PE
            # transpose back into e1's left half.
            e01 = vpool.tile([128, 128], F32, tag="e01", name="e01")
            nc.scalar.copy(out=e01[:], in_=e0[:, 128:256])
            nc.tensor.transpose(e1[:, 0:128], e01[:], ident[:])

            w_sb = []
            for h in range(2):
                e_ps = st["e_ps"][h]
                mn = vpool.tile([128, 1], F32, tag=f"mn{h}", name=f"mn{h}")
                nc.vector.tensor_reduce(
                    out=mn[:], in_=e_ps[:],
                    op=mybir.AluOpType.min, axis=mybir.AxisListType.X,
                )
                u = upool.tile([128, c], BF16, tag=f"u{h}", name=f"u{h}")
                z = vpool.tile([128, 1], F32, tag=f"z{h}", name=f"z{h}")
                nc.scalar.activation(
                    out=u[:], in_=e_ps[:],
                    func=mybir.ActivationFunctionType.Exp,
                    bias=mn[:], scale=-1.0, accum_out=z[:],
                )
                rz = vpool.tile([128, 1], F32, tag=f"r{h}", name=f"rz{h}")
                nc.vector.reciprocal(out=rz[:], in_=z[:])
                s = vpool.tile([128, 1], F32, tag=f"s{h}", name=f"s{h}")
                nc.vector.tensor_mul(s[:], rz[:], alpha_b[:])
                # W = (alpha/Z) * U, with +identity on the diagonal block:
                # out = (W + I) @ x then needs no epilogue at all.
                w = wpool.tile([128, c], BF16, tag=f"w{h}", name=f"w{h}")
                od = 128 - h * 128  # off-diagonal block offset
                nc.vector.scalar_tensor_tensor(
                    out=w[:, h * 128:h * 128 + 128],
                    in0=u[:, h * 128:h * 128 + 128],
                    scalar=s[:], in1=identb[:],
                    op0=mybir.AluOpType.mult, op1=mybir.AluOpType.add,
                )
                nc.vector.tensor_scalar_mul(
                    out=w[:, od:od + 128], in0=u[:, od:od + 128], scalar1=s[:]
                )
                w_sb.append(w)

            wt_sb = []
            for jc in range(2):
                utp = tp_psum.tile([128, 4 * c], BF16, tag="tp", name="utp")
                for h in range(2):
                    nc.tensor.transpose(
                        utp[:, h * 128:(h + 1) * 128],
                        w_sb[h][:, jc * 128:(jc + 1) * 128],
                        identb[:],
                    )
                wt = utpool.tile([128, c], BF16, tag=f"ut{jc}", name=f"ut{jc}")
                # ACT: at the sample boundary DVE is busy with the next
                # sample's xt copies; ACT is idle right after the exp.
                nc.scalar.copy(out=wt[:], in_=utp[:, :c])
                wt_sb.append(wt)
            st["wt_sb"] = wt_sb

        def out_gen(st):
            """Yields after each 1024-wide output column chunk."""
            b, xn = st["b"], st["xn"]
            wt_sb = st["wt_sb"]
            stgs = [None, None]
            # last sample: halve the staging span so the final stores
            # overlap the copies instead of draining after them.
            lnst = nst // 2 if b == nsamp - 1 else nst

            for pc in range(nout):
                for h in range(2):
                    if pc % lnst == 0:
                        stgs[h] = opool.tile(
                            [128, lnst * 1024], BF16, tag=f"st{h}",
                            name=f"stg{h}"
                        )
                    o_ps = o_psum.tile([128, 1024], F32, tag="o", name="o_ps")
                    for ph in range(2):
                        # matmul PSUM dst must stay within one 2KB bank:
                        # write the two 512-col halves separately.
                        for jc in range(2):
                            nc.tensor.matmul(
                                o_ps[:, ph * 512:(ph + 1) * 512],
                                lhsT=wt_sb[jc][:, h * 128:(h + 1) * 128],
                                rhs=xn[jc][:, pc * 1024 + ph * 512:
                                           pc * 1024 + (ph + 1) * 512],
                                start=(jc == 0),
                                stop=(jc == 1),
                            )
                    dst = stgs[h][:, (pc % lnst) * 1024:(pc % lnst + 1) * 1024]
                    # alternate the PSUM-drain copies between DVE and ACT
                    # so neither engine's copy cadence gates the PE.
                    if (pc + h) % 2 == 0:
                        nc.vector.tensor_copy(out=dst, in_=o_ps[:])
                    else:
                        nc.scalar.copy(out=dst, in_=o_ps[:])
                    if pc % lnst == lnst - 1:
                        c0 = (pc - lnst + 1) * 1024
                        nc.sync.dma_start(
                            out=o_d[b, h * 128:(h + 1) * 128,
                                    c0:c0 + lnst * 1024],
                            in_=stgs[h][:],
                        )
                yield

        def drain(gen):
            for _ in gen:
                pass

        # --- pipeline driver ---
        drain(energy_gen(st_cur))
        emit_softmax_ut(st_cur)
        for b in range(nsamp):
            st_nxt = None
            eg = None
            if b + 1 < nsamp:
                st_nxt = emit_load(b + 1)
                eg = energy_gen(st_nxt)
            og = out_gen(st_cur)
            # +1: the generator's final segment (last emm) sits past its
            # last yield, so budget one extra next() to reach the done
            # path while out chunks remain to hide the softmax under.
            ratio = max(1, (nunits + nout) // nout) + 1
            for _ in og:
                if eg is not None:
                    done = False
                    for _ in range(ratio):
                        if next(eg, StopIteration) is StopIteration:
                            done = True
                            break
                    if done:
                        # energy(b+1) fully emitted: slot its softmax + W^T
                        # under the remaining out(b) chunks so the sample
                        # boundary has no PE bubble.
                        emit_softmax_ut(st_nxt)
                        eg = None
                        st_cur = st_nxt
                        st_nxt = None
            if eg is not None:
                drain(eg)
                emit_softmax_ut(st_nxt)
                st_cur = st_nxt

    nc.compile()
    return nc


_NC_CACHE = {}


def _get_nc(nsamp=FULL_B // N_CORES, c=C, p=P):
    key = (nsamp, c, p)
    if key not in _NC_CACHE:
        _NC_CACHE[key] = build(nsamp, c, p)
    return _NC_CACHE[key]


def _run(x, alpha, trace=False):
    x = np.ascontiguousarray(np.asarray(x, dtype=np.float32))
    alpha = np.ascontiguousarray(np.asarray(alpha, dtype=np.float32))
    assert x.shape == (FULL_B, C, P), x.shape
    ns = FULL_B // N_CORES
    nc = _get_nc()
    in_maps = [
        {"x": x[ci * ns:(ci + 1) * ns], "alpha": alpha} for ci in range(N_CORES)
    ]
    res = run_bass_kernel_spmd(
        nc, in_maps, list(range(N_CORES)), trace=trace,
    )
    out = np.concatenate(
        [
            np.asarray(res.results[ci]["out"]).astype(np.float32)
            for ci in range(N_CORES)
        ],
        axis=0,
    )
    return out, res


def kernel(x, alpha):
    out, _ = _run(x, alpha, trace=False)
    return out
